# revision 5
# baseline (speedup 1.0000x reference)
"""DepthRouteNet Trainium2 kernel: 8-core data-parallel Bass/Tile implementation.

Layout: feature-major activations [feat->partitions, batch->free].
Math: fp32r (tf32) matmuls everywhere except the final gate matmul (fp32,
since top-k selection boundaries need <5e-5 logit error), routing in fp32.
Per core: batch 1024, processed in two half-batches of 512 columns.

Self-contained: hardcodes all shapes; no file reads.
"""
import numpy as np

import concourse.bacc as bacc
import concourse.mybir as mybir
import concourse.tile as tile
from concourse.bass import ds
from concourse.masks import make_identity

F32 = mybir.dt.float32
F32R = mybir.dt.float32r
AX = mybir.AxisListType
ALU = mybir.AluOpType
ACT_F = mybir.ActivationFunctionType

NCORES = 8
B = 8192
BC = B // NCORES          # 1024 per core
HALF = BC // 2            # 512
IN, H, EMH, GH, OUT = 64, 1024, 256, 512, 18
HC = H // 128             # 8 feature chunks
GOUT = 10
NEG = -1e30

_CACHE = {}


def _mm_layer(nc, ps, lhsT, rhs, kc, m_chunks, m_size, out_cb, tag, f32=False):
    """out = lhsT.T @ rhs per m-chunk, accumulating over kc K-chunks.

    lhsT: [<=128, kc, M_total] tile; rhs: [<=128, kc, N] tile view.
    out_cb(m, psum_ap) consumes the [m_size, N] PSUM result.
    """
    n = rhs.shape[-1]
    for m in range(m_chunks):
        pt = ps.tile([128, 512], F32, tag=tag)
        pv = pt[:m_size, :n]
        for k in range(kc):
            nc.tensor.matmul(
                pv,
                lhsT[:, k, ds(m * m_size, m_size)],
                rhs[:, k] if rhs.ndim == 3 else rhs,
                start=(k == 0),
                stop=(k == kc - 1),
            )
        out_cb(m, pv)


def build():
    nc = bacc.Bacc(debug=False)

    # ---- DRAM inputs
    xT_d = nc.dram_tensor("xT", [IN, BC], F32, kind="ExternalInput")
    emT_d = nc.dram_tensor("emT", [1, BC], F32, kind="ExternalInput")
    bW0_d = nc.dram_tensor("bW0", [IN, H], F32, kind="ExternalInput")
    bW1_d = nc.dram_tensor("bW1", [H, H], F32, kind="ExternalInput")
    eW0_d = nc.dram_tensor("eW0", [1, EMH], F32, kind="ExternalInput")
    eW1_d = nc.dram_tensor("eW1", [EMH, H], F32, kind="ExternalInput")
    gW0_d = nc.dram_tensor("gW0", [H, GH], F32, kind="ExternalInput")
    gW1_d = nc.dram_tensor("gW1", [GH, GOUT], F32, kind="ExternalInput")
    fcW_d = nc.dram_tensor("fcW", [4, H, H], F32, kind="ExternalInput")
    lW_d = nc.dram_tensor("lW", [H, OUT], F32, kind="ExternalInput")
    bb0_d = nc.dram_tensor("bb0", [128, HC], F32, kind="ExternalInput")
    bb1_d = nc.dram_tensor("bb1", [128, HC], F32, kind="ExternalInput")
    eb0_d = nc.dram_tensor("eb0", [128, 2], F32, kind="ExternalInput")
    eb1_d = nc.dram_tensor("eb1", [128, HC], F32, kind="ExternalInput")
    gb0_d = nc.dram_tensor("gb0", [128, 4], F32, kind="ExternalInput")
    gb1_d = nc.dram_tensor("gb1", [GOUT, 1], F32, kind="ExternalInput")
    fcb_d = nc.dram_tensor("fcb", [128, 4, HC], F32, kind="ExternalInput")
    lb_d = nc.dram_tensor("lb", [OUT, 1], F32, kind="ExternalInput")

    # ---- DRAM outputs (batch-major per core)
    final_d = nc.dram_tensor("final", [BC, OUT], F32, kind="ExternalOutput")
    gates_d = nc.dram_tensor("gates", [BC, GOUT], F32, kind="ExternalOutput")
    onehot_d = nc.dram_tensor("onehot", [BC, GOUT], F32, kind="ExternalOutput")
    soft_d = nc.dram_tensor("soft", [BC, GOUT], F32, kind="ExternalOutput")

    with tile.TileContext(nc) as tc:
        with (
            tc.tile_pool(name="const", bufs=1) as cp,
            tc.tile_pool(name="persist", bufs=1) as pp,
            tc.tile_pool(name="fcw", bufs=1) as fwp,
            tc.tile_pool(name="psmm", bufs=4, space="PSUM") as ps,
            tc.tile_pool(name="pssm", bufs=2, space="PSUM") as ps2,
        ):
            # constants / biases
            ident = cp.tile([128, 128], F32)
            make_identity(nc, ident[:])
            bb0 = cp.tile([128, HC], F32); nc.sync.dma_start(bb0[:], bb0_d[:])
            bb1 = cp.tile([128, HC], F32); nc.sync.dma_start(bb1[:], bb1_d[:])
            eb0 = cp.tile([128, 2], F32); nc.sync.dma_start(eb0[:], eb0_d[:])
            eb1 = cp.tile([128, HC], F32); nc.sync.dma_start(eb1[:], eb1_d[:])
            gb0 = cp.tile([128, 4], F32); nc.sync.dma_start(gb0[:], gb0_d[:])
            gb1 = cp.tile([GOUT, 1], F32); nc.sync.dma_start(gb1[:], gb1_d[:])
            fcb = cp.tile([128, 4, HC], F32); nc.sync.dma_start(fcb[:], fcb_d[:])
            lb = cp.tile([OUT, 1], F32); nc.sync.dma_start(lb[:], lb_d[:])

            # persistent tensors
            xt = pp.tile([IN, BC], F32R)
            nc.sync.dma_start(xt[:], xT_d[:].bitcast(F32R))
            emt = pp.tile([1, BC], F32R)
            nc.sync.dma_start(emt[:], emT_d[:].bitcast(F32R))
            lw = pp.tile([128, HC, OUT], F32R)
            nc.sync.dma_start(lw[:], lW_d[:].rearrange("(kc p) m -> p kc m", p=128).bitcast(F32R))
            gw1 = pp.tile([128, 4, GOUT], F32)
            nc.sync.dma_start(gw1[:], gW1_d[:].rearrange("(kc p) m -> p kc m", p=128))
            mi = pp.tile([128, HC, BC], F32R, tag="mi")          # module_input, both halves
            logitsT = pp.tile([GOUT, BC], F32)
            gatesT = pp.tile([GOUT, BC], F32)

            # prefetch chain W for layer 0 early (overlaps prefix compute)
            fcw_tiles = {}
            fcw0 = fwp.tile([128, HC, H], F32R, tag="fcw")
            nc.sync.dma_start(fcw0[:], fcW_d[0].rearrange("(kc p) m -> p kc m", p=128).bitcast(F32R))
            fcw_tiles[(0, 0)] = fcw0

            # ================= PHASE 1: prefix MLPs =================
            with tc.tile_pool(name="pw", bufs=1) as pw, tc.tile_pool(name="a1", bufs=1) as a1:
                bw0 = pw.tile([IN, H], F32R)
                nc.sync.dma_start(bw0[:], bW0_d[:].bitcast(F32R))
                bw1 = pw.tile([128, HC, H], F32R)
                nc.sync.dma_start(bw1[:], bW1_d[:].rearrange("(kc p) m -> p kc m", p=128).bitcast(F32R))
                ew0 = pw.tile([1, EMH], F32R)
                nc.sync.dma_start(ew0[:], eW0_d[:].bitcast(F32R))
                ew1 = pw.tile([128, 2, H], F32R)
                nc.sync.dma_start(ew1[:], eW1_d[:].rearrange("(kc p) m -> p kc m", p=128).bitcast(F32R))
                gw0 = pw.tile([128, HC, GH], F32R)
                nc.sync.dma_start(gw0[:], gW0_d[:].rearrange("(kc p) m -> p kc m", p=128).bitcast(F32R))

                for h in range(2):
                    hs = ds(h * HALF, HALF)
                    # base layer 0: h1 = relu(bW0.T @ xT + bb0)
                    h1 = a1.tile([128, HC, HALF], F32R, tag="h1")
                    _mm_layer(nc, ps, bw0.unsqueeze(1), xt[:, hs], 1, HC, 128,
                              lambda m, pv: nc.scalar.activation(h1[:, m], pv, ACT_F.Relu, bias=bb0[:, m:m + 1]),
                              tag="mm")
                    # base layer 1: base_out = bW1.T @ h1 + bb1
                    bo = a1.tile([128, HC, HALF], F32, tag="bo")
                    _mm_layer(nc, ps, bw1, h1, HC, HC, 128,
                              lambda m, pv: nc.scalar.activation(bo[:, m], pv, ACT_F.Identity, bias=bb1[:, m:m + 1]),
                              tag="mm")
                    # em layer 0: eh = relu(eW0.T @ emT + eb0)
                    eh = a1.tile([128, 2, HALF], F32R, tag="eh")
                    _mm_layer(nc, ps, ew0.unsqueeze(1), emt[:, hs], 1, 2, 128,
                              lambda m, pv: nc.scalar.activation(eh[:, m], pv, ACT_F.Relu, bias=eb0[:, m:m + 1]),
                              tag="mm")
                    # em layer 1: e = relu(eW1.T @ eh + eb1)
                    e = a1.tile([128, HC, HALF], F32R, tag="e")
                    _mm_layer(nc, ps, ew1, eh, 2, HC, 128,
                              lambda m, pv: nc.scalar.activation(e[:, m], pv, ACT_F.Relu, bias=eb1[:, m:m + 1]),
                              tag="mm")
                    # gate_in = e * base_out (in place over e, rounded to f32r)
                    for c in range(HC):
                        nc.vector.tensor_tensor(e[:, c], e[:, c].bitcast(F32), bo[:, c], op=ALU.mult)
                    # module_input = relu(base_out)
                    for c in range(HC):
                        nc.scalar.activation(mi[:, c, hs], bo[:, c], ACT_F.Relu)
                    # gate layer 0: gh = relu(gW0.T @ gate_in + gb0)   (gh fp32 for the fp32 g1)
                    gh = a1.tile([128, 4, HALF], F32, tag="gh")
                    _mm_layer(nc, ps, gw0, e, HC, 4, 128,
                              lambda m, pv: nc.scalar.activation(gh[:, m], pv, ACT_F.Relu, bias=gb0[:, m:m + 1]),
                              tag="mm")
                    # gate layer 1 (fp32): logits = gW1.T @ gh + gb1
                    pl = ps2.tile([128, 512], F32, tag="sm")
                    plv = pl[:GOUT, :HALF]
                    for k in range(4):
                        nc.tensor.matmul(plv, gw1[:, k], gh[:, k], start=(k == 0), stop=(k == 3))
                    nc.scalar.activation(logitsT[:, hs], plv, ACT_F.Identity, bias=gb1[:])

            # ================= ROUTING (batch-major) =================
            with tc.tile_pool(name="rt", bufs=1) as rt:
                NG = BC // 128  # 8 groups
                Lb = rt.tile([128, NG, GOUT], F32)
                for g in range(NG):
                    pt = ps2.tile([128, 512], F32, tag="sm")
                    ptv = pt[:, :GOUT]
                    nc.tensor.transpose(ptv, logitsT[:, ds(g * 128, 128)], ident[:GOUT, :GOUT])
                    nc.scalar.copy(Lb[:, g], ptv)
                gates_b = rt.tile([128, NG, GOUT], F32)
                onehot_b = rt.tile([128, NG, GOUT], F32)
                soft_b = rt.tile([128, NG, GOUT], F32)
                eL = rt.tile([128, NG, GOUT], F32)
                nc.scalar.activation(eL[:], Lb[:], ACT_F.Exp)
                red = rt.tile([128, NG, 1], F32, tag="red")
                red2 = rt.tile([128, NG, 1], F32, tag="red2")
                tmp4 = rt.tile([128, NG, 4], F32, tag="tmp4")
                lm2f = rt.tile([128, NG, 4], F32, tag="lm2f")
                for off, m in [(0, 1), (1, 2), (3, 3), (6, 4)]:
                    Ls = Lb[:, :, off:off + m]
                    es = eL[:, :, off:off + m]
                    g_ = gates_b[:, :, off:off + m]
                    oh = onehot_b[:, :, off:off + m]
                    so = soft_b[:, :, off:off + m]
                    if m == 1:
                        nc.vector.memset(g_, 1.0)
                        nc.vector.memset(oh, 1.0)
                        nc.vector.memset(so, 1.0)
                        continue
                    s = red[:, :, :]
                    nc.vector.reduce_sum(s, es, axis=AX.X)
                    r = red2[:, :, :]
                    nc.vector.reciprocal(r, s)
                    nc.vector.tensor_tensor(so, es, r.broadcast_to([128, NG, m]), op=ALU.mult)
                    if m == 2:
                        nc.vector.memset(oh, 1.0)
                        nc.vector.tensor_copy(g_, so)
                        continue
                    v1 = red[:, :, :]
                    nc.vector.reduce_max(v1, Ls, axis=AX.X)
                    m1 = tmp4[:, :, :m]
                    nc.vector.tensor_tensor(m1, Ls, v1.broadcast_to([128, NG, m]), op=ALU.is_ge)
                    lm2 = lm2f[:, :, :m]
                    nc.vector.scalar_tensor_tensor(lm2, m1, NEG, Ls, op0=ALU.mult, op1=ALU.add)
                    v2 = red[:, :, :]
                    nc.vector.reduce_max(v2, lm2, axis=AX.X)
                    nc.vector.tensor_tensor(oh, Ls, v2.broadcast_to([128, NG, m]), op=ALU.is_ge)
                    pe2 = tmp4[:, :, :m]
                    nc.vector.tensor_tensor(pe2, es, oh, op=ALU.mult)
                    s2 = red[:, :, :]
                    nc.vector.reduce_sum(s2, pe2, axis=AX.X)
                    r2 = red2[:, :, :]
                    nc.vector.reciprocal(r2, s2)
                    nc.vector.tensor_tensor(g_, pe2, r2.broadcast_to([128, NG, m]), op=ALU.mult)

                nc.sync.dma_start(gates_d[:].rearrange("(g p) t -> p g t", p=128), gates_b[:])
                nc.sync.dma_start(onehot_d[:].rearrange("(g p) t -> p g t", p=128), onehot_b[:])
                nc.sync.dma_start(soft_d[:].rearrange("(g p) t -> p g t", p=128), soft_b[:])

                # gates back to feature-major [10, BC]
                for g in range(NG):
                    pt = ps2.tile([128, 512], F32, tag="sm")
                    ptv = pt[:GOUT, :128]
                    nc.tensor.transpose(ptv, gates_b[:, g], ident[:])
                    nc.scalar.copy(gatesT[:, ds(g * 128, 128)], ptv)

            # ================= PHASE 2: module chain =================
            with tc.tile_pool(name="ch", bufs=1) as ch:
                for h in range(2):
                    hs = ds(h * HALF, HALF)
                    outs = []
                    # layer 0
                    if (h, 0) not in fcw_tiles:
                        t = fwp.tile([128, HC, H], F32R, tag="fcw")
                        nc.sync.dma_start(t[:], fcW_d[0].rearrange("(kc p) m -> p kc m", p=128).bitcast(F32R))
                        fcw_tiles[(h, 0)] = t
                    out0 = ch.tile([128, HC, HALF], F32R, tag="out0")
                    _mm_layer(nc, ps, fcw_tiles[(h, 0)], mi[:, :, hs], HC, HC, 128,
                              lambda m, pv: nc.scalar.activation(out0[:, m], pv, ACT_F.Relu, bias=fcb[:, 0, m:m + 1]),
                              tag="mm")
                    outs.append(out0)
                    for lev in range(1, 4):
                        # prefetch weights
                        t = fwp.tile([128, HC, H], F32R, tag="fcw")
                        nc.sync.dma_start(t[:], fcW_d[lev].rearrange("(kc p) m -> p kc m", p=128).bitcast(F32R))
                        # mix: acc = sum_j gates[goff+j] * outs[j]
                        # (level 1's gate is the trivial m=1 split == 1.0, so inp = out0)
                        if lev == 1:
                            acc = outs[0]
                        else:
                            acc = ch.tile([128, HC, HALF], F32R, tag="acc")
                            self_mix(nc, ch, ps, gatesT, hs, outs, lev, acc)
                        outl = ch.tile([128, HC, HALF], F32R, tag=f"out{lev}")
                        _mm_layer(nc, ps, t, acc, HC, HC, 128,
                                  lambda m, pv, _l=lev, _o=outl: nc.scalar.activation(
                                      _o[:, m], pv, ACT_F.Relu, bias=fcb[:, _l, m:m + 1]),
                                  tag="mm")
                        outs.append(outl)
                    # final mix (split 4) + last layer
                    facc = ch.tile([128, HC, HALF], F32R, tag="acc")
                    self_mix(nc, ch, ps, gatesT, hs, outs, 4, facc)
                    pf = ps2.tile([128, 512], F32, tag="sm")
                    pfv = pf[:OUT, :HALF]
                    for k in range(HC):
                        nc.tensor.matmul(pfv, lw[:, k], facc[:, k], start=(k == 0), stop=(k == HC - 1))
                    fT = ch.tile([OUT, HALF], F32, tag="fT")
                    nc.scalar.activation(fT[:], pfv, ACT_F.Identity, bias=lb[:])
                    # transpose back to batch-major and store
                    for g in range(HALF // 128):
                        pt = ps2.tile([128, 512], F32, tag="sm")
                        ptv = pt[:, :OUT]
                        nc.tensor.transpose(ptv, fT[:, ds(g * 128, 128)], ident[:OUT, :OUT])
                        fb = ch.tile([128, OUT], F32, tag="fb")
                        nc.scalar.copy(fb[:], ptv)
                        nc.sync.dma_start(
                            final_d[:].rearrange("(G p) t -> p G t", p=128)[:, h * (HALF // 128) + g],
                            fb[:])

    nc.compile()
    return nc


def self_mix(nc, ch, ps, gatesT, hs, outs, msize, acc):
    """acc[:, k] = sum_j bcast(gates[goff+j]) * outs[j][:, k]  (split of size msize)."""
    goff = {2: 1, 3: 3, 4: 6}[msize]
    nterms = len(outs)
    bcasts = []
    for j in range(nterms):
        stage = ch.tile([1, HALF], F32, tag="stage", bufs=2)
        nc.sync.dma_start(stage[:], gatesT[goff + j:goff + j + 1, hs])
        bc = ch.tile([128, HALF], F32, tag="bc", bufs=5)
        nc.gpsimd.partition_broadcast(bc[:], stage[:])
        bcasts.append(bc)
    for k in range(HC):
        nc.vector.tensor_tensor(acc[:, k], outs[0][:, k].bitcast(F32), bcasts[0][:], op=ALU.mult)
        for j in range(1, nterms):
            tmp = ch.tile([128, HALF], F32, tag="mtmp", bufs=2)
            nc.vector.tensor_tensor(tmp[:], outs[j][:, k].bitcast(F32), bcasts[j][:], op=ALU.mult)
            nc.vector.tensor_tensor(acc[:, k], acc[:, k].bitcast(F32), tmp[:], op=ALU.add)


def _prep_host(inputs):
    """Shared (per-core-identical) weight arrays, prepped for the kernel."""
    f = lambda a: np.ascontiguousarray(np.asarray(a, dtype=np.float32))
    chunk = lambda b, c: np.ascontiguousarray(np.asarray(b, np.float32).reshape(c, 128).T)
    w = {
        "bW0": f(inputs["base_W0"]), "bW1": f(inputs["base_W1"]),
        "eW0": f(inputs["em_W0"]), "eW1": f(inputs["em_W1"]),
        "gW0": f(inputs["gate_W0"]), "gW1": f(inputs["gate_W1"]),
        "fcW": f(inputs["fc_W"]), "lW": f(inputs["last_W"]),
        "bb0": chunk(inputs["base_b0"], HC), "bb1": chunk(inputs["base_b1"], HC),
        "eb0": chunk(inputs["em_b0"], 2), "eb1": chunk(inputs["em_b1"], HC),
        "gb0": chunk(inputs["gate_b0"], 4),
        "gb1": f(inputs["gate_b1"]).reshape(GOUT, 1),
        "fcb": np.ascontiguousarray(
            np.asarray(inputs["fc_b"], np.float32).reshape(4, HC, 128).transpose(2, 0, 1)),
        "lb": f(inputs["last_b"]).reshape(OUT, 1),
    }
    return w


def kernel(**inputs):
    if "nc" not in _CACHE:
        _CACHE["nc"] = build()
    nc = _CACHE["nc"]

    w = _prep_host(inputs)
    x = np.asarray(inputs["x"], np.float32)
    em = np.asarray(inputs["em"], np.float32)
    in_maps = []
    for c in range(NCORES):
        sl = slice(c * BC, (c + 1) * BC)
        m = dict(w)
        m["xT"] = np.ascontiguousarray(x[sl].T)
        m["emT"] = np.ascontiguousarray(em[sl].reshape(1, BC))
        in_maps.append(m)

    from concourse.bass_utils import run_bass_kernel_spmd
    res = run_bass_kernel_spmd(nc, in_maps, core_ids=list(range(NCORES)))
    _CACHE["last_results"] = res

    final = np.concatenate([res.results[c]["final"] for c in range(NCORES)], axis=0)
    gates = np.concatenate([res.results[c]["gates"] for c in range(NCORES)], axis=0)
    onehot = np.concatenate([res.results[c]["onehot"] for c in range(NCORES)], axis=0)
    soft = np.concatenate([res.results[c]["soft"] for c in range(NCORES)], axis=0)
    return final, gates, onehot, soft


# revision 6
# speedup vs baseline: 2.3072x; 2.3072x over previous
"""DepthRouteNet Trainium2 kernel: 8-core data-parallel Bass/Tile implementation.

Layout: feature-major activations [feat->partitions, batch->free].
Math: fp32r (tf32) matmuls everywhere except the final gate matmul (fp32,
since top-k selection boundaries need <5e-5 logit error), routing in fp32.
Per core: batch 1024, processed in two half-batches of 512 columns.

Self-contained: hardcodes all shapes; no file reads.
"""
import numpy as np

import concourse.bacc as bacc
import concourse.mybir as mybir
import concourse.tile as tile
from concourse.bass import ds
from concourse.masks import make_identity

F32 = mybir.dt.float32
F32R = mybir.dt.float32r
AX = mybir.AxisListType
ALU = mybir.AluOpType
ACT_F = mybir.ActivationFunctionType

NCORES = 8
B = 8192
BC = B // NCORES          # 1024 per core
HALF = BC // 2            # 512
IN, H, EMH, GH, OUT = 64, 1024, 256, 512, 18
HC = H // 128             # 8 feature chunks
GOUT = 10
NEG = -1e30

_CACHE = {}


def _mm_layer(nc, ps, lhsT, rhs, kc, m_chunks, m_size, out_cb, tag, f32=False):
    """out = lhsT.T @ rhs per m-chunk, accumulating over kc K-chunks.

    lhsT: [<=128, kc, M_total] tile; rhs: [<=128, kc, N] tile view.
    out_cb(m, psum_ap) consumes the [m_size, N] PSUM result.
    """
    n = rhs.shape[-1]
    for m in range(m_chunks):
        pt = ps.tile([128, 512], F32, tag=tag)
        pv = pt[:m_size, :n]
        for k in range(kc):
            nc.tensor.matmul(
                pv,
                lhsT[:, k, ds(m * m_size, m_size)],
                rhs[:, k] if rhs.ndim == 3 else rhs,
                start=(k == 0),
                stop=(k == kc - 1),
            )
        out_cb(m, pv)


def build():
    nc = bacc.Bacc(debug=False)

    # ---- DRAM inputs
    xT_d = nc.dram_tensor("xT", [IN, BC], F32, kind="ExternalInput")
    emT_d = nc.dram_tensor("emT", [1, BC], F32, kind="ExternalInput")
    bW0_d = nc.dram_tensor("bW0", [IN, H], F32, kind="ExternalInput")
    bW1_d = nc.dram_tensor("bW1", [H, H], F32, kind="ExternalInput")
    eW0_d = nc.dram_tensor("eW0", [1, EMH], F32, kind="ExternalInput")
    eW1_d = nc.dram_tensor("eW1", [EMH, H], F32, kind="ExternalInput")
    gW0_d = nc.dram_tensor("gW0", [H, GH], F32, kind="ExternalInput")
    gW1_d = nc.dram_tensor("gW1", [GH, GOUT], F32, kind="ExternalInput")
    fcW_d = nc.dram_tensor("fcW", [4, H, H], F32, kind="ExternalInput")
    lW_d = nc.dram_tensor("lW", [H, OUT], F32, kind="ExternalInput")
    bb0_d = nc.dram_tensor("bb0", [128, HC], F32, kind="ExternalInput")
    bb1_d = nc.dram_tensor("bb1", [128, HC], F32, kind="ExternalInput")
    eb0_d = nc.dram_tensor("eb0", [128, 2], F32, kind="ExternalInput")
    eb1_d = nc.dram_tensor("eb1", [128, HC], F32, kind="ExternalInput")
    gb0_d = nc.dram_tensor("gb0", [128, 4], F32, kind="ExternalInput")
    gb1_d = nc.dram_tensor("gb1", [GOUT, 1], F32, kind="ExternalInput")
    fcb_d = nc.dram_tensor("fcb", [128, 4, HC], F32, kind="ExternalInput")
    lb_d = nc.dram_tensor("lb", [OUT, 1], F32, kind="ExternalInput")

    # ---- DRAM outputs (batch-major per core)
    final_d = nc.dram_tensor("final", [BC, OUT], F32, kind="ExternalOutput")
    gates_d = nc.dram_tensor("gates", [BC, GOUT], F32, kind="ExternalOutput")
    onehot_d = nc.dram_tensor("onehot", [BC, GOUT], F32, kind="ExternalOutput")
    soft_d = nc.dram_tensor("soft", [BC, GOUT], F32, kind="ExternalOutput")

    with tile.TileContext(nc) as tc:
        with (
            tc.tile_pool(name="const", bufs=1) as cp,
            tc.tile_pool(name="persist", bufs=1) as pp,
            tc.tile_pool(name="fcw", bufs=1) as fwp,
            tc.tile_pool(name="psmm", bufs=4, space="PSUM") as ps,
            tc.tile_pool(name="pssm", bufs=2, space="PSUM") as ps2,
        ):
            # constants / biases
            ident = cp.tile([128, 128], F32)
            make_identity(nc, ident[:])
            bb0 = cp.tile([128, HC], F32); nc.sync.dma_start(bb0[:], bb0_d[:])
            bb1 = cp.tile([128, HC], F32); nc.sync.dma_start(bb1[:], bb1_d[:])
            eb0 = cp.tile([128, 2], F32); nc.sync.dma_start(eb0[:], eb0_d[:])
            eb1 = cp.tile([128, HC], F32); nc.sync.dma_start(eb1[:], eb1_d[:])
            gb0 = cp.tile([128, 4], F32); nc.sync.dma_start(gb0[:], gb0_d[:])
            gb1 = cp.tile([GOUT, 1], F32); nc.sync.dma_start(gb1[:], gb1_d[:])
            fcb = cp.tile([128, 4, HC], F32); nc.sync.dma_start(fcb[:], fcb_d[:])
            lb = cp.tile([OUT, 1], F32); nc.sync.dma_start(lb[:], lb_d[:])

            # persistent tensors
            xt = pp.tile([IN, BC], F32R)
            nc.sync.dma_start(xt[:], xT_d[:].bitcast(F32R))
            emt = pp.tile([1, BC], F32R)
            nc.sync.dma_start(emt[:], emT_d[:].bitcast(F32R))
            lw = pp.tile([128, HC, OUT], F32R)
            nc.sync.dma_start(lw[:], lW_d[:].rearrange("(kc p) m -> p kc m", p=128).bitcast(F32R))
            gw1 = pp.tile([128, 4, GOUT], F32)
            nc.sync.dma_start(gw1[:], gW1_d[:].rearrange("(kc p) m -> p kc m", p=128))
            mi = pp.tile([128, HC, BC], F32R, tag="mi")          # module_input, both halves
            logitsT = pp.tile([GOUT, BC], F32)
            gatesT = pp.tile([GOUT, BC], F32)

            # prefetch chain W for layer 0 early (overlaps prefix compute)
            fcw_tiles = {}
            fcw0 = fwp.tile([128, HC, H], F32R, tag="fcw")
            nc.sync.dma_start(fcw0[:], fcW_d[0].rearrange("(kc p) m -> p kc m", p=128).bitcast(F32R))
            fcw_tiles[(0, 0)] = fcw0

            # ================= PHASE 1: prefix MLPs =================
            with tc.tile_pool(name="pw", bufs=1) as pw, tc.tile_pool(name="a1", bufs=1) as a1:
                bw0 = pw.tile([IN, H], F32R)
                nc.sync.dma_start(bw0[:], bW0_d[:].bitcast(F32R))
                bw1 = pw.tile([128, HC, H], F32R)
                nc.sync.dma_start(bw1[:], bW1_d[:].rearrange("(kc p) m -> p kc m", p=128).bitcast(F32R))
                ew0 = pw.tile([1, EMH], F32R)
                nc.sync.dma_start(ew0[:], eW0_d[:].bitcast(F32R))
                ew1 = pw.tile([128, 2, H], F32R)
                nc.sync.dma_start(ew1[:], eW1_d[:].rearrange("(kc p) m -> p kc m", p=128).bitcast(F32R))
                gw0 = pw.tile([128, HC, GH], F32R)
                nc.sync.dma_start(gw0[:], gW0_d[:].rearrange("(kc p) m -> p kc m", p=128).bitcast(F32R))

                for h in range(2):
                    hs = ds(h * HALF, HALF)
                    # base layer 0: h1 = relu(bW0.T @ xT + bb0)
                    h1 = a1.tile([128, HC, HALF], F32R, tag="h1")
                    _mm_layer(nc, ps, bw0.unsqueeze(1), xt[:, hs], 1, HC, 128,
                              lambda m, pv: nc.scalar.activation(h1[:, m], pv, ACT_F.Relu, bias=bb0[:, m:m + 1]),
                              tag="mm")
                    # base layer 1: base_out = bW1.T @ h1 + bb1
                    bo = a1.tile([128, HC, HALF], F32, tag="bo")
                    _mm_layer(nc, ps, bw1, h1, HC, HC, 128,
                              lambda m, pv: nc.scalar.activation(bo[:, m], pv, ACT_F.Identity, bias=bb1[:, m:m + 1]),
                              tag="mm")
                    # em layer 0: eh = relu(eW0.T @ emT + eb0)
                    eh = a1.tile([128, 2, HALF], F32R, tag="eh")
                    _mm_layer(nc, ps, ew0.unsqueeze(1), emt[:, hs], 1, 2, 128,
                              lambda m, pv: nc.scalar.activation(eh[:, m], pv, ACT_F.Relu, bias=eb0[:, m:m + 1]),
                              tag="mm")
                    # em layer 1: e = relu(eW1.T @ eh + eb1)
                    e = a1.tile([128, HC, HALF], F32R, tag="e")
                    _mm_layer(nc, ps, ew1, eh, 2, HC, 128,
                              lambda m, pv: nc.scalar.activation(e[:, m], pv, ACT_F.Relu, bias=eb1[:, m:m + 1]),
                              tag="mm")
                    # gate_in = e * base_out (in place over e, rounded to f32r)
                    for c in range(HC):
                        nc.vector.tensor_tensor(e[:, c], e[:, c].bitcast(F32), bo[:, c], op=ALU.mult)
                    # module_input = relu(base_out)
                    for c in range(HC):
                        nc.scalar.activation(mi[:, c, hs], bo[:, c], ACT_F.Relu)
                    # gate layer 0: gh = relu(gW0.T @ gate_in + gb0)   (gh fp32 for the fp32 g1)
                    gh = a1.tile([128, 4, HALF], F32, tag="gh")
                    _mm_layer(nc, ps, gw0, e, HC, 4, 128,
                              lambda m, pv: nc.scalar.activation(gh[:, m], pv, ACT_F.Relu, bias=gb0[:, m:m + 1]),
                              tag="mm")
                    # gate layer 1 (fp32): logits = gW1.T @ gh + gb1
                    pl = ps2.tile([128, 512], F32, tag="sm")
                    plv = pl[:GOUT, :HALF]
                    for k in range(4):
                        nc.tensor.matmul(plv, gw1[:, k], gh[:, k], start=(k == 0), stop=(k == 3))
                    nc.scalar.activation(logitsT[:, hs], plv, ACT_F.Identity, bias=gb1[:])

            # ================= ROUTING (batch-major) =================
            with tc.tile_pool(name="rt", bufs=1) as rt:
                NG = BC // 128  # 8 groups
                Lb = rt.tile([128, NG, GOUT], F32)
                for g in range(NG):
                    pt = ps2.tile([128, 512], F32, tag="sm")
                    ptv = pt[:, :GOUT]
                    nc.tensor.transpose(ptv, logitsT[:, ds(g * 128, 128)], ident[:GOUT, :GOUT])
                    nc.scalar.copy(Lb[:, g], ptv)
                gates_b = rt.tile([128, NG, GOUT], F32)
                onehot_b = rt.tile([128, NG, GOUT], F32)
                soft_b = rt.tile([128, NG, GOUT], F32)
                eL = rt.tile([128, NG, GOUT], F32)
                nc.scalar.activation(eL[:], Lb[:], ACT_F.Exp)
                red = rt.tile([128, NG, 1], F32, tag="red")
                red2 = rt.tile([128, NG, 1], F32, tag="red2")
                tmp4 = rt.tile([128, NG, 4], F32, tag="tmp4")
                lm2f = rt.tile([128, NG, 4], F32, tag="lm2f")
                for off, m in [(0, 1), (1, 2), (3, 3), (6, 4)]:
                    Ls = Lb[:, :, off:off + m]
                    es = eL[:, :, off:off + m]
                    g_ = gates_b[:, :, off:off + m]
                    oh = onehot_b[:, :, off:off + m]
                    so = soft_b[:, :, off:off + m]
                    if m == 1:
                        nc.vector.memset(g_, 1.0)
                        nc.vector.memset(oh, 1.0)
                        nc.vector.memset(so, 1.0)
                        continue
                    s = red[:, :, :]
                    nc.vector.reduce_sum(s, es, axis=AX.X)
                    r = red2[:, :, :]
                    nc.vector.reciprocal(r, s)
                    nc.vector.tensor_tensor(so, es, r.broadcast_to([128, NG, m]), op=ALU.mult)
                    if m == 2:
                        nc.vector.memset(oh, 1.0)
                        nc.vector.tensor_copy(g_, so)
                        continue
                    v1 = red[:, :, :]
                    nc.vector.reduce_max(v1, Ls, axis=AX.X)
                    m1 = tmp4[:, :, :m]
                    nc.vector.tensor_tensor(m1, Ls, v1.broadcast_to([128, NG, m]), op=ALU.is_ge)
                    lm2 = lm2f[:, :, :m]
                    nc.vector.scalar_tensor_tensor(lm2, m1, NEG, Ls, op0=ALU.mult, op1=ALU.add)
                    v2 = red[:, :, :]
                    nc.vector.reduce_max(v2, lm2, axis=AX.X)
                    nc.vector.tensor_tensor(oh, Ls, v2.broadcast_to([128, NG, m]), op=ALU.is_ge)
                    pe2 = tmp4[:, :, :m]
                    nc.vector.tensor_tensor(pe2, es, oh, op=ALU.mult)
                    s2 = red[:, :, :]
                    nc.vector.reduce_sum(s2, pe2, axis=AX.X)
                    r2 = red2[:, :, :]
                    nc.vector.reciprocal(r2, s2)
                    nc.vector.tensor_tensor(g_, pe2, r2.broadcast_to([128, NG, m]), op=ALU.mult)

                nc.sync.dma_start(gates_d[:].rearrange("(g p) t -> p g t", p=128), gates_b[:])
                nc.sync.dma_start(onehot_d[:].rearrange("(g p) t -> p g t", p=128), onehot_b[:])
                nc.sync.dma_start(soft_d[:].rearrange("(g p) t -> p g t", p=128), soft_b[:])

                # gates back to feature-major [10, BC]
                for g in range(NG):
                    pt = ps2.tile([128, 512], F32, tag="sm")
                    ptv = pt[:GOUT, :128]
                    nc.tensor.transpose(ptv, gates_b[:, g], ident[:])
                    nc.scalar.copy(gatesT[:, ds(g * 128, 128)], ptv)

            # ================= PHASE 2: module chain =================
            with tc.tile_pool(name="ch", bufs=1) as ch:
                for h in range(2):
                    hs = ds(h * HALF, HALF)
                    outs = []
                    # layer 0
                    if (h, 0) not in fcw_tiles:
                        t = fwp.tile([128, HC, H], F32R, tag="fcw")
                        nc.sync.dma_start(t[:], fcW_d[0].rearrange("(kc p) m -> p kc m", p=128).bitcast(F32R))
                        fcw_tiles[(h, 0)] = t
                    out0 = ch.tile([128, HC, HALF], F32R, tag="out0")
                    _mm_layer(nc, ps, fcw_tiles[(h, 0)], mi[:, :, hs], HC, HC, 128,
                              lambda m, pv: nc.scalar.activation(out0[:, m], pv, ACT_F.Relu, bias=fcb[:, 0, m:m + 1]),
                              tag="mm")
                    outs.append(out0)
                    for lev in range(1, 4):
                        # prefetch weights
                        t = fwp.tile([128, HC, H], F32R, tag="fcw")
                        nc.sync.dma_start(t[:], fcW_d[lev].rearrange("(kc p) m -> p kc m", p=128).bitcast(F32R))
                        # mix: acc = sum_j gates[goff+j] * outs[j]
                        # (level 1's gate is the trivial m=1 split == 1.0, so inp = out0)
                        if lev == 1:
                            acc = outs[0]
                        else:
                            acc = ch.tile([128, HC, HALF], F32R, tag="acc")
                            self_mix(nc, ch, ps, gatesT, hs, outs, lev, acc)
                        outl = ch.tile([128, HC, HALF], F32R, tag=f"out{lev}")
                        _mm_layer(nc, ps, t, acc, HC, HC, 128,
                                  lambda m, pv, _l=lev, _o=outl: nc.scalar.activation(
                                      _o[:, m], pv, ACT_F.Relu, bias=fcb[:, _l, m:m + 1]),
                                  tag="mm")
                        outs.append(outl)
                    # final mix (split 4) + last layer
                    facc = ch.tile([128, HC, HALF], F32R, tag="acc")
                    self_mix(nc, ch, ps, gatesT, hs, outs, 4, facc)
                    pf = ps2.tile([128, 512], F32, tag="sm")
                    pfv = pf[:OUT, :HALF]
                    for k in range(HC):
                        nc.tensor.matmul(pfv, lw[:, k], facc[:, k], start=(k == 0), stop=(k == HC - 1))
                    fT = ch.tile([OUT, HALF], F32, tag="fT")
                    nc.scalar.activation(fT[:], pfv, ACT_F.Identity, bias=lb[:])
                    # transpose back to batch-major and store
                    for g in range(HALF // 128):
                        pt = ps2.tile([128, 512], F32, tag="sm")
                        ptv = pt[:, :OUT]
                        nc.tensor.transpose(ptv, fT[:, ds(g * 128, 128)], ident[:OUT, :OUT])
                        fb = ch.tile([128, OUT], F32, tag="fb")
                        nc.scalar.copy(fb[:], ptv)
                        nc.sync.dma_start(
                            final_d[:].rearrange("(G p) t -> p G t", p=128)[:, h * (HALF // 128) + g],
                            fb[:])

    nc.compile()
    return nc


def self_mix(nc, ch, ps, gatesT, hs, outs, msize, acc):
    """acc[:, k] = sum_j bcast(gates[goff+j]) * outs[j][:, k]  (split of size msize)."""
    goff = {2: 1, 3: 3, 4: 6}[msize]
    nterms = len(outs)
    bcasts = []
    for j in range(nterms):
        stage = ch.tile([1, HALF], F32, tag="stage", bufs=2)
        nc.sync.dma_start(stage[:], gatesT[goff + j:goff + j + 1, hs])
        bc = ch.tile([128, HALF], F32, tag="bc", bufs=5)
        nc.gpsimd.partition_broadcast(bc[:], stage[:])
        bcasts.append(bc)
    for k in range(HC):
        nc.vector.tensor_tensor(acc[:, k], outs[0][:, k].bitcast(F32), bcasts[0][:], op=ALU.mult)
        for j in range(1, nterms):
            tmp = ch.tile([128, HALF], F32, tag="mtmp", bufs=2)
            nc.vector.tensor_tensor(tmp[:], outs[j][:, k].bitcast(F32), bcasts[j][:], op=ALU.mult)
            nc.vector.tensor_tensor(acc[:, k], acc[:, k].bitcast(F32), tmp[:], op=ALU.add)


def _prep_host(inputs):
    """Shared (per-core-identical) weight arrays, prepped for the kernel."""
    f = lambda a: np.ascontiguousarray(np.asarray(a, dtype=np.float32))
    chunk = lambda b, c: np.ascontiguousarray(np.asarray(b, np.float32).reshape(c, 128).T)
    w = {
        "bW0": f(inputs["base_W0"]), "bW1": f(inputs["base_W1"]),
        "eW0": f(inputs["em_W0"]), "eW1": f(inputs["em_W1"]),
        "gW0": f(inputs["gate_W0"]), "gW1": f(inputs["gate_W1"]),
        "fcW": f(inputs["fc_W"]), "lW": f(inputs["last_W"]),
        "bb0": chunk(inputs["base_b0"], HC), "bb1": chunk(inputs["base_b1"], HC),
        "eb0": chunk(inputs["em_b0"], 2), "eb1": chunk(inputs["em_b1"], HC),
        "gb0": chunk(inputs["gate_b0"], 4),
        "gb1": f(inputs["gate_b1"]).reshape(GOUT, 1),
        "fcb": np.ascontiguousarray(
            np.asarray(inputs["fc_b"], np.float32).reshape(4, HC, 128).transpose(2, 0, 1)),
        "lb": f(inputs["last_b"]).reshape(OUT, 1),
    }
    return w


def kernel(**inputs):
    if "nc" not in _CACHE:
        _CACHE["nc"] = build()
    nc = _CACHE["nc"]

    w = _prep_host(inputs)
    x = np.asarray(inputs["x"], np.float32)
    em = np.asarray(inputs["em"], np.float32)
    in_maps = []
    for c in range(NCORES):
        sl = slice(c * BC, (c + 1) * BC)
        m = dict(w)
        m["xT"] = np.ascontiguousarray(x[sl].T)
        m["emT"] = np.ascontiguousarray(em[sl].reshape(1, BC))
        in_maps.append(m)

    import os
    os.environ.setdefault("BASS_NEVER_TRACE", "1")  # axon NTFF hook absent here
    from concourse.bass_utils import run_bass_kernel_spmd
    res = run_bass_kernel_spmd(nc, in_maps, core_ids=list(range(NCORES)))
    _CACHE["last_results"] = res

    final = np.concatenate([res.results[c]["final"] for c in range(NCORES)], axis=0)
    gates = np.concatenate([res.results[c]["gates"] for c in range(NCORES)], axis=0)
    onehot = np.concatenate([res.results[c]["onehot"] for c in range(NCORES)], axis=0)
    soft = np.concatenate([res.results[c]["soft"] for c in range(NCORES)], axis=0)
    return final, gates, onehot, soft
